# revision 1
# baseline (speedup 1.0000x reference)
"""Positional-encoding add kernel for Trainium2 (8 NeuronCores, SPMD).

Problem: X[4, 4096, 2048] f32; out = X + PE[None, :, :] where
  PE[s, 2i]   = sin(s / 10000^(2i/2048))
  PE[s, 2i+1] = cos(s / 10000^(2i/2048))

Sharding: sequence dim split 8 ways -> 512 positions per core.
Per core the shard is [4, 512, 2048] = 16 MiB, flattened to rows
[2048, 2048] (row = b*512 + s_local).  The 4 MiB PE shard for the
core's 512 positions is loaded once into SBUF and reused for all 4
batches; X streams through in 4 MiB [128, 8192] tiles (one batch
each, partition p holding positions 4p..4p+3).

This is purely memory-bound: 32 MiB X traffic + 4 MiB PE per core.
"""

import os

import numpy as np

B, S, D = 4, 4096, 2048
N_CORES = 8
S_SHARD = S // N_CORES          # 512 positions per core
ROWS = B * S_SHARD              # 2048 rows per core
P = 128                         # SBUF partitions
R = S_SHARD // P                # 4 positions per partition
FREE = R * D                    # 8192 free elems per partition

_cached_nc = None
LAST_RESULT = None              # BassKernelResults of the last run (for test.py)


def _build_nc(repeat: int = 1):
    import concourse.bacc as bacc
    import concourse.mybir as mybir
    from concourse.tile import TileContext

    f32 = mybir.dt.float32
    nc = bacc.Bacc(None, target_bir_lowering=False, debug=False)
    x = nc.dram_tensor("X", [ROWS, D], f32, kind="ExternalInput")
    pe = nc.dram_tensor("PE", [S_SHARD, D], f32, kind="ExternalInput")
    out = nc.dram_tensor("OUT", [ROWS, D], f32, kind="ExternalOutput")

    # 1 MiB tiles: tile t covers rows [t*128, (t+1)*128); row = 512*b + s_local,
    # so tile t is batch t//4, position block (t%4)*128, matching PE tile t%4.
    n_tiles = ROWS // P          # 16
    n_pe = S_SHARD // P          # 4
    xv = x.rearrange("(t p) d -> t p d", t=n_tiles, p=P)
    ov = out.rearrange("(t p) d -> t p d", t=n_tiles, p=P)
    pev = pe.rearrange("(t p) d -> t p d", t=n_pe, p=P)

    with TileContext(nc) as tc:
        with (
            tc.tile_pool(name="pe", bufs=n_pe) as pe_pool,
            tc.tile_pool(name="xs", bufs=16) as xs_pool,
        ):
            pe_ts = []
            for t in range(n_pe):
                pt = pe_pool.tile([P, D], f32)
                # SWDGE ring for PE so the sync ring starts X loads at t=0
                nc.gpsimd.dma_start(out=pt, in_=pev[t])
                pe_ts.append(pt)
            for _rep in range(repeat):
                for t in range(n_tiles):
                    xt = xs_pool.tile([P, D], f32)
                    nc.sync.dma_start(out=xt, in_=xv[t])
                    # fp32 tensor_tensor runs at 1x on DVE (no 2x uop); offload
                    # every 3rd add to GpSimd (~2x slower) to balance engines
                    eng = nc.gpsimd if t % 3 == 2 else nc.vector
                    eng.tensor_add(out=xt, in0=xt, in1=pe_ts[t % n_pe])
                    nc.sync.dma_start(out=ov[t], in_=xt)
    nc.finalize()
    return nc


def _pe_table() -> np.ndarray:
    """PE table [S, D] f32, matching the jax-on-CPU f32 reference bitwise."""
    try:
        import jax

        with jax.default_device(jax.devices("cpu")[0]):
            import jax.numpy as jnp

            pos = jnp.arange(S, dtype=jnp.float32)[:, None]
            i = jnp.arange(D // 2, dtype=jnp.float32)[None, :]
            angle = pos / jnp.power(jnp.asarray(10000.0, jnp.float32), 2.0 * i / D)
            pe = jnp.stack([jnp.sin(angle), jnp.cos(angle)], axis=-1)
            return np.asarray(pe.reshape(S, D), dtype=np.float32)
    except Exception:
        pos = np.arange(S, dtype=np.float32)[:, None]
        i = np.arange(D // 2, dtype=np.float32)[None, :]
        expo = ((np.float32(2.0) * i) / np.float32(D)).astype(np.float32)
        denom = np.power(np.float32(10000.0), expo, dtype=np.float32)
        angle = (pos / denom).astype(np.float32)
        pe = np.stack(
            [np.sin(angle, dtype=np.float32), np.cos(angle, dtype=np.float32)],
            axis=-1,
        )
        return np.ascontiguousarray(pe.reshape(S, D), dtype=np.float32)


def kernel(X: np.ndarray) -> np.ndarray:
    global _cached_nc, LAST_RESULT
    from concourse.bass_utils import run_bass_kernel_spmd

    X = np.asarray(X)
    assert X.shape == (B, S, D), X.shape
    X = np.ascontiguousarray(X, dtype=np.float32)

    if _cached_nc is None:
        _cached_nc = _build_nc()
    nc = _cached_nc

    pe = _pe_table()
    in_maps = []
    for c in range(N_CORES):
        xs = np.ascontiguousarray(X[:, c * S_SHARD : (c + 1) * S_SHARD, :]).reshape(
            ROWS, D
        )
        pes = np.ascontiguousarray(pe[c * S_SHARD : (c + 1) * S_SHARD, :])
        in_maps.append({"X": xs, "PE": pes})

    trace = bool(int(os.environ.get("KERNEL_TRACE", "0")))
    res = run_bass_kernel_spmd(
        nc, in_maps, core_ids=list(range(N_CORES)), trace=trace
    )
    LAST_RESULT = res

    out = np.empty((B, S, D), dtype=np.float32)
    for c in range(N_CORES):
        out[:, c * S_SHARD : (c + 1) * S_SHARD, :] = res.results[c]["OUT"].reshape(
            B, S_SHARD, D
        )
    return out



# revision 2
# speedup vs baseline: 1.6967x; 1.6967x over previous
"""Positional-encoding add kernel for Trainium2 (8 NeuronCores, SPMD).

Problem: X[4, 4096, 2048] f32; out = X + PE[None, :, :] where
  PE[s, 2i]   = sin(s / 10000^(2i/2048))
  PE[s, 2i+1] = cos(s / 10000^(2i/2048))

Sharding: sequence dim split 8 ways -> 512 positions per core.
Per core the shard is [4, 512, 2048] = 16 MiB, flattened to rows
[2048, 2048] (row = b*512 + s_local).

Phased design: concurrent HBM reads+writes on trn2 run at ~310 GB/s/core
while one-directional streams hit ~416 GB/s (measured). So: load the
whole 16 MiB shard + 4 MiB PE into SBUF (reads only), add in place
(DVE, overlapped with the tail of the load phase), all-engine barrier,
then store the whole 16 MiB (writes only). ~20 MiB in + 16 MiB out at
~410 GB/s/dir beats 36 MiB mixed at ~310.
"""

import os

import numpy as np

B, S, D = 4, 4096, 2048
N_CORES = 8
S_SHARD = S // N_CORES          # 512 positions per core
ROWS = B * S_SHARD              # 2048 rows per core
P = 128                         # SBUF partitions
N_TILES = ROWS // P             # 16 x [128, 2048] 1 MiB tiles
N_PE = S_SHARD // P             # 4 PE blocks [128, 2048]

_cached_nc = None
LAST_RESULT = None              # BassKernelResults of the last run (for test.py)


def _build_nc(repeat: int = 1):
    import concourse.bacc as bacc
    import concourse.mybir as mybir
    from concourse.tile import TileContext

    f32 = mybir.dt.float32
    nc = bacc.Bacc(None, target_bir_lowering=False, debug=False)
    x = nc.dram_tensor("X", [ROWS, D], f32, kind="ExternalInput")
    pe = nc.dram_tensor("PE", [S_SHARD, D], f32, kind="ExternalInput")
    out = nc.dram_tensor("OUT", [ROWS, D], f32, kind="ExternalOutput")

    # tile t covers rows [t*128, (t+1)*128); row = 512*b + s_local,
    # so tile t is batch t//4, position block (t%4)*128 -> PE block t%4.
    xv = x.rearrange("(t p) d -> t p d", t=N_TILES, p=P)
    ov = out.rearrange("(t p) d -> t p d", t=N_TILES, p=P)
    pev = pe.rearrange("(t p) d -> t p d", t=N_PE, p=P)

    with TileContext(nc) as tc:
        with (
            tc.tile_pool(name="pe", bufs=1) as pe_pool,
            tc.tile_pool(name="xs", bufs=1) as xs_pool,
        ):
            pe_ts = [
                pe_pool.tile([P, D], f32, name=f"pe{t}") for t in range(N_PE)
            ]
            x_ts = [
                xs_pool.tile([P, D], f32, name=f"x{t}") for t in range(N_TILES)
            ]
            # PE via SWDGE ring so the sync ring starts X loads at t=0;
            # all phase-1 traffic is HBM reads.
            for t in range(N_PE):
                nc.gpsimd.dma_start(out=pe_ts[t], in_=pev[t])
            for _rep in range(repeat):
                for t in range(N_TILES):
                    nc.sync.dma_start(out=x_ts[t], in_=xv[t])
                for t in range(N_TILES):
                    nc.vector.tensor_add(
                        out=x_ts[t], in0=x_ts[t], in1=pe_ts[t % N_PE]
                    )
                # phase boundary: no store descriptor moves until all
                # loads+adds are done, keeping HBM one-directional.
                tc.strict_bb_all_engine_barrier()
                for t in range(N_TILES):
                    nc.sync.dma_start(out=ov[t], in_=x_ts[t])
                tc.strict_bb_all_engine_barrier()
    nc.finalize()
    return nc


def _pe_table() -> np.ndarray:
    """PE table [S, D] f32, matching the jax-on-CPU f32 reference bitwise."""
    try:
        import jax

        with jax.default_device(jax.devices("cpu")[0]):
            import jax.numpy as jnp

            pos = jnp.arange(S, dtype=jnp.float32)[:, None]
            i = jnp.arange(D // 2, dtype=jnp.float32)[None, :]
            angle = pos / jnp.power(jnp.asarray(10000.0, jnp.float32), 2.0 * i / D)
            pe = jnp.stack([jnp.sin(angle), jnp.cos(angle)], axis=-1)
            return np.asarray(pe.reshape(S, D), dtype=np.float32)
    except Exception:
        pos = np.arange(S, dtype=np.float32)[:, None]
        i = np.arange(D // 2, dtype=np.float32)[None, :]
        expo = ((np.float32(2.0) * i) / np.float32(D)).astype(np.float32)
        denom = np.power(np.float32(10000.0), expo, dtype=np.float32)
        angle = (pos / denom).astype(np.float32)
        pe = np.stack(
            [np.sin(angle, dtype=np.float32), np.cos(angle, dtype=np.float32)],
            axis=-1,
        )
        return np.ascontiguousarray(pe.reshape(S, D), dtype=np.float32)


def kernel(X: np.ndarray) -> np.ndarray:
    global _cached_nc, LAST_RESULT
    from concourse.bass_utils import run_bass_kernel_spmd

    X = np.asarray(X)
    assert X.shape == (B, S, D), X.shape
    X = np.ascontiguousarray(X, dtype=np.float32)

    if _cached_nc is None:
        _cached_nc = _build_nc()
    nc = _cached_nc

    pe = _pe_table()
    in_maps = []
    for c in range(N_CORES):
        xs = np.ascontiguousarray(X[:, c * S_SHARD : (c + 1) * S_SHARD, :]).reshape(
            ROWS, D
        )
        pes = np.ascontiguousarray(pe[c * S_SHARD : (c + 1) * S_SHARD, :])
        in_maps.append({"X": xs, "PE": pes})

    trace = bool(int(os.environ.get("KERNEL_TRACE", "0")))
    res = run_bass_kernel_spmd(
        nc, in_maps, core_ids=list(range(N_CORES)), trace=trace
    )
    LAST_RESULT = res

    out = np.empty((B, S, D), dtype=np.float32)
    for c in range(N_CORES):
        out[:, c * S_SHARD : (c + 1) * S_SHARD, :] = res.results[c]["OUT"].reshape(
            B, S_SHARD, D
        )
    return out


# revision 3
# speedup vs baseline: 1.7500x; 1.0314x over previous
"""Positional-encoding add kernel for Trainium2 (8 NeuronCores, SPMD).

Problem: X[4, 4096, 2048] f32; out = X + PE[None, :, :] where
  PE[s, 2i]   = sin(s / 10000^(2i/2048))
  PE[s, 2i+1] = cos(s / 10000^(2i/2048))

Sharding: sequence dim split 8 ways -> 512 positions per core.
Per core the shard is [4, 512, 2048] = 16 MiB, flattened to rows
[2048, 2048] (row = b*512 + s_local).

Phased design: concurrent HBM reads+writes on trn2 run at ~310 GB/s/core
while one-directional streams hit ~416 GB/s (measured). So: load the
whole 16 MiB shard + 4 MiB PE into SBUF (reads only), add in place
(DVE, overlapped with the tail of the load phase), all-engine barrier,
then store the whole 16 MiB (writes only). ~20 MiB in + 16 MiB out at
~410 GB/s/dir beats 36 MiB mixed at ~310.
"""

import os

import numpy as np

B, S, D = 4, 4096, 2048
N_CORES = 8
S_SHARD = S // N_CORES          # 512 positions per core
ROWS = B * S_SHARD              # 2048 rows per core
P = 128                         # SBUF partitions
N_TILES = ROWS // P             # 16 x [128, 2048] 1 MiB tiles
N_PE = S_SHARD // P             # 4 PE blocks [128, 2048]

_cached_nc = None
LAST_RESULT = None              # BassKernelResults of the last run (for test.py)


PE_BF16 = True


def _build_nc(repeat: int = 1):
    import concourse.bacc as bacc
    import concourse.mybir as mybir
    from concourse.tile import TileContext

    f32 = mybir.dt.float32
    pe_dt = mybir.dt.bfloat16 if PE_BF16 else f32
    nc = bacc.Bacc(None, target_bir_lowering=False, debug=False)
    x = nc.dram_tensor("X", [ROWS, D], f32, kind="ExternalInput")
    pe = nc.dram_tensor("PE", [S_SHARD, D], pe_dt, kind="ExternalInput")
    out = nc.dram_tensor("OUT", [ROWS, D], f32, kind="ExternalOutput")

    # tile t covers rows [t*128, (t+1)*128); row = 512*b + s_local,
    # so tile t is batch t//4, position block (t%4)*128 -> PE block t%4.
    xv = x.rearrange("(t p) d -> t p d", t=N_TILES, p=P)
    ov = out.rearrange("(t p) d -> t p d", t=N_TILES, p=P)
    pev = pe.rearrange("(t p) d -> t p d", t=N_PE, p=P)

    with TileContext(nc) as tc:
        with (
            tc.tile_pool(name="pe", bufs=1) as pe_pool,
            tc.tile_pool(name="xs", bufs=1) as xs_pool,
        ):
            pe_ts = [
                pe_pool.tile([P, D], pe_dt, name=f"pe{t}") for t in range(N_PE)
            ]
            x_ts = [
                xs_pool.tile([P, D], f32, name=f"x{t}") for t in range(N_TILES)
            ]
            # PE via SWDGE ring so the sync ring starts X loads at t=0;
            # all phase-1 traffic is HBM reads.
            for t in range(N_PE):
                nc.gpsimd.dma_start(out=pe_ts[t], in_=pev[t])
            for _rep in range(repeat):
                for t in range(N_TILES):
                    nc.sync.dma_start(out=x_ts[t], in_=xv[t])
                for t in range(N_TILES):
                    nc.vector.tensor_add(
                        out=x_ts[t], in0=x_ts[t], in1=pe_ts[t % N_PE]
                    )
                # phase boundary: no store descriptor moves until all
                # loads+adds are done, keeping HBM one-directional.
                tc.strict_bb_all_engine_barrier()
                for t in range(N_TILES):
                    nc.sync.dma_start(out=ov[t], in_=x_ts[t])
                tc.strict_bb_all_engine_barrier()
    nc.finalize()
    return nc


def _pe_table() -> np.ndarray:
    """PE table [S, D] f32, matching the jax-on-CPU f32 reference bitwise."""
    try:
        import jax

        with jax.default_device(jax.devices("cpu")[0]):
            import jax.numpy as jnp

            pos = jnp.arange(S, dtype=jnp.float32)[:, None]
            i = jnp.arange(D // 2, dtype=jnp.float32)[None, :]
            angle = pos / jnp.power(jnp.asarray(10000.0, jnp.float32), 2.0 * i / D)
            pe = jnp.stack([jnp.sin(angle), jnp.cos(angle)], axis=-1)
            return np.asarray(pe.reshape(S, D), dtype=np.float32)
    except Exception:
        pos = np.arange(S, dtype=np.float32)[:, None]
        i = np.arange(D // 2, dtype=np.float32)[None, :]
        expo = ((np.float32(2.0) * i) / np.float32(D)).astype(np.float32)
        denom = np.power(np.float32(10000.0), expo, dtype=np.float32)
        angle = (pos / denom).astype(np.float32)
        pe = np.stack(
            [np.sin(angle, dtype=np.float32), np.cos(angle, dtype=np.float32)],
            axis=-1,
        )
        return np.ascontiguousarray(pe.reshape(S, D), dtype=np.float32)


def kernel(X: np.ndarray) -> np.ndarray:
    global _cached_nc, LAST_RESULT
    from concourse.bass_utils import run_bass_kernel_spmd

    X = np.asarray(X)
    assert X.shape == (B, S, D), X.shape
    X = np.ascontiguousarray(X, dtype=np.float32)

    if _cached_nc is None:
        _cached_nc = _build_nc()
    nc = _cached_nc

    pe = _pe_table()
    in_maps = []
    for c in range(N_CORES):
        xs = np.ascontiguousarray(X[:, c * S_SHARD : (c + 1) * S_SHARD, :]).reshape(
            ROWS, D
        )
        pes = np.ascontiguousarray(pe[c * S_SHARD : (c + 1) * S_SHARD, :])
        if PE_BF16:
            import ml_dtypes

            pes = pes.astype(ml_dtypes.bfloat16)
        in_maps.append({"X": xs, "PE": pes})

    trace = bool(int(os.environ.get("KERNEL_TRACE", "0")))
    res = run_bass_kernel_spmd(
        nc, in_maps, core_ids=list(range(N_CORES)), trace=trace
    )
    LAST_RESULT = res

    out = np.empty((B, S, D), dtype=np.float32)
    for c in range(N_CORES):
        out[:, c * S_SHARD : (c + 1) * S_SHARD, :] = res.results[c]["OUT"].reshape(
            B, S_SHARD, D
        )
    return out


# revision 4
# speedup vs baseline: 1.7782x; 1.0161x over previous
"""Positional-encoding add kernel for Trainium2 (8 NeuronCores, SPMD).

Problem: X[4, 4096, 2048] f32; out = X + PE[None, :, :] where
  PE[s, 2i]   = sin(s / 10000^(2i/2048))
  PE[s, 2i+1] = cos(s / 10000^(2i/2048))

Sharding: sequence dim split 8 ways -> 512 positions per core.
Per core the shard is [4, 512, 2048] = 16 MiB, flattened to rows
[2048, 2048] (row = b*512 + s_local).

Design notes (HW-measured):
- Per-NC HBM bandwidth is ~358 GB/s per direction; concurrent reads+
  writes degrade to ~310 GB/s. So the kernel is phased: load everything
  (reads only), add in place, then store (writes only). Phase separation
  falls out of the single HWDGE ring's FIFO order - no barriers needed:
  stores enqueue behind all loads, so directions never interleave.
- The PE table is generated on device (saves 4 MiB/core of HBM traffic;
  only a 512 KiB invf table + 2 KiB positions come from HBM):
    a  = invf * pos                     (DVE, per-partition scalar)
    k  = round(a / 2pi)  [+0.25 for cos] (ACT Identity with i32 output,
                                          convert rounds to nearest)
    m  = a - 2pi*k  in [-pi, pi]         (DVE scalar_tensor_tensor)
    pe = Sin(m)  /  Sin(m2 + pi/2)       (ACT; Sin is accurate in-range)
  Max PE error vs the f32 reference table: ~5e-4 (tolerance is 2e-2).
- Steady-state body ~= (16.5 MiB + 16 MiB) / 358 GB/s ~= 95 us/core.
"""

import os

import numpy as np

B, S, D = 4, 4096, 2048
N_CORES = 8
S_SHARD = S // N_CORES          # 512 positions per core
ROWS = B * S_SHARD              # 2048 rows per core
P = 128                         # SBUF partitions
K = D // 2                      # 1024 frequencies
N_TILES = ROWS // P             # 16 x [128, 2048] 1 MiB tiles
N_PE = S_SHARD // P             # 4 PE blocks [128, 2048]

_cached_nc = None
LAST_RESULT = None              # BassKernelResults of the last run (for test.py)


def _build_nc(repeat: int = 1):
    import concourse.bacc as bacc
    import concourse.mybir as mybir
    from concourse.tile import TileContext

    f32 = mybir.dt.float32
    i32 = mybir.dt.int32
    PI = float(np.pi)
    Sin = mybir.ActivationFunctionType.Sin
    Ident = mybir.ActivationFunctionType.Identity
    A = mybir.AluOpType

    nc = bacc.Bacc(None, target_bir_lowering=False, debug=False)
    x = nc.dram_tensor("X", [ROWS, D], f32, kind="ExternalInput")
    invf = nc.dram_tensor("INVF", [P, K], f32, kind="ExternalInput")
    pos = nc.dram_tensor("POS", [P, N_PE], f32, kind="ExternalInput")
    out = nc.dram_tensor("OUT", [ROWS, D], f32, kind="ExternalOutput")

    # tile t covers rows [t*128, (t+1)*128); row = 512*b + s_local,
    # so tile t is batch t//4, position block (t%4)*128 -> PE block t%4.
    xv = x.rearrange("(t p) d -> t p d", t=N_TILES, p=P)
    ov = out.rearrange("(t p) d -> t p d", t=N_TILES, p=P)

    with TileContext(nc) as tc:
        with (
            tc.tile_pool(name="pe", bufs=1) as pe_pool,
            tc.tile_pool(name="xs", bufs=1) as xs_pool,
            tc.tile_pool(name="gen", bufs=1) as gen_pool,
        ):
            pe_ts = [
                pe_pool.tile([P, D], f32, name=f"pe{t}") for t in range(N_PE)
            ]
            x_ts = [
                xs_pool.tile([P, D], f32, name=f"x{t}") for t in range(N_TILES)
            ]
            invf_t = gen_pool.tile([P, K], f32, name="invf_t")
            pos_t = gen_pool.tile([P, N_PE], f32, name="pos_t")
            pi2 = gen_pool.tile([P, 1], f32, name="pi2")
            qtr = gen_pool.tile([P, 1], f32, name="qtr")
            a_ts = [gen_pool.tile([P, K], f32, name=f"a{b}") for b in range(N_PE)]
            m_ts = [gen_pool.tile([P, K], f32, name=f"m{b}") for b in range(N_PE)]
            k1_t = gen_pool.tile([P, K], i32, name="k1_t")
            k2_t = gen_pool.tile([P, K], i32, name="k2_t")

            # tiny PE-gen inputs on the ACT ring; X loads own the sync ring
            nc.scalar.dma_start(out=invf_t, in_=invf[:, :])
            nc.scalar.dma_start(out=pos_t, in_=pos[:, :])
            nc.vector.memset(pi2, PI / 2)
            nc.vector.memset(qtr, 0.25)

            # on-device PE: block b holds positions pos_t[:, b]
            for b in range(N_PE):
                a = a_ts[b]
                m2 = m_ts[b]
                nc.vector.tensor_scalar(
                    out=a, in0=invf_t, scalar1=pos_t[:, b : b + 1],
                    scalar2=None, op0=A.mult)
                nc.scalar.activation(out=k1_t, in_=a, func=Ident,
                                     scale=float(1 / (2 * PI)))
                nc.scalar.activation(out=k2_t, in_=a, func=Ident,
                                     scale=float(1 / (2 * PI)),
                                     bias=qtr[:, 0:1])
                nc.vector.scalar_tensor_tensor(
                    out=m2, in0=k2_t, scalar=float(-2 * PI), in1=a,
                    op0=A.mult, op1=A.add)
                nc.scalar.activation(out=pe_ts[b][:, 1::2], in_=m2, func=Sin,
                                     bias=pi2[:, 0:1])
                nc.vector.scalar_tensor_tensor(
                    out=a, in0=k1_t, scalar=float(-2 * PI), in1=a,
                    op0=A.mult, op1=A.add)
                nc.scalar.activation(out=pe_ts[b][:, 0::2], in_=a, func=Sin)

            for _rep in range(repeat):
                for t in range(N_TILES):
                    nc.sync.dma_start(out=x_ts[t], in_=xv[t])
                for t in range(N_TILES):
                    nc.vector.tensor_add(
                        out=x_ts[t], in0=x_ts[t], in1=pe_ts[t % N_PE]
                    )
                # stores enqueue on the same ring behind all loads: the
                # FIFO keeps HBM one-directional in each phase.
                for t in range(N_TILES):
                    nc.sync.dma_start(out=ov[t], in_=x_ts[t])
    nc.finalize()
    return nc


def _invf_row() -> np.ndarray:
    """invf[k] = 1 / 10000^(2k/D) as f32, matching the jax f32 reference."""
    try:
        import jax

        with jax.default_device(jax.devices("cpu")[0]):
            import jax.numpy as jnp

            i = jnp.arange(K, dtype=jnp.float32)[None, :]
            denom = jnp.power(jnp.asarray(10000.0, jnp.float32), 2.0 * i / D)
            return np.asarray(1.0 / denom, dtype=np.float32).reshape(K)
    except Exception:
        i = np.arange(K, dtype=np.float32)
        expo = ((np.float32(2.0) * i) / np.float32(D)).astype(np.float32)
        denom = np.power(np.float32(10000.0), expo, dtype=np.float32)
        return (np.float32(1.0) / denom).astype(np.float32)


def _in_maps(X: np.ndarray) -> list:
    """Per-core input dicts for the SPMD kernel."""
    invf = np.ascontiguousarray(
        np.broadcast_to(_invf_row()[None, :], (P, K)), dtype=np.float32
    )
    in_maps = []
    for c in range(N_CORES):
        xs = np.ascontiguousarray(
            X[:, c * S_SHARD : (c + 1) * S_SHARD, :]
        ).reshape(ROWS, D)
        p = np.arange(P, dtype=np.float32)[:, None]
        b = np.arange(N_PE, dtype=np.float32)[None, :]
        posc = (c * S_SHARD + b * P + p).astype(np.float32)
        in_maps.append({"X": xs, "INVF": invf, "POS": posc})
    return in_maps


def kernel(X: np.ndarray) -> np.ndarray:
    global _cached_nc, LAST_RESULT
    from concourse.bass_utils import run_bass_kernel_spmd

    X = np.asarray(X)
    assert X.shape == (B, S, D), X.shape
    X = np.ascontiguousarray(X, dtype=np.float32)

    if _cached_nc is None:
        _cached_nc = _build_nc()
    nc = _cached_nc

    trace = bool(int(os.environ.get("KERNEL_TRACE", "0")))
    res = run_bass_kernel_spmd(
        nc, _in_maps(X), core_ids=list(range(N_CORES)), trace=trace
    )
    LAST_RESULT = res

    out = np.empty((B, S, D), dtype=np.float32)
    for c in range(N_CORES):
        out[:, c * S_SHARD : (c + 1) * S_SHARD, :] = res.results[c]["OUT"].reshape(
            B, S_SHARD, D
        )
    return out
